# revision 10
# baseline (speedup 1.0000x reference)
"""VQ codebook forward (nn_Codebook) on 8 Trainium2 NeuronCores.

Strategy (data-parallel over tokens, weights replicated):
  - 65536 tokens sharded 8192/core (2 full batches per core, so the
    per-batch losses stay core-local).
  - All big matmuls run as bf16 hi/lo 3-term products accumulated in
    f32 PSUM (error ~1e-5 absolute, fp32-grade; validated 0 argmin
    flips vs the f32 reference on the real data).
  - proj-down produces z_e_down TRANSPOSED ([e, t]) so it can feed the
    scores matmul as the PE stationary operand with contraction e on
    partitions; the host un-transposes the [256, 8192] output.
  - scores m[t,k] = <zd,c> - ||c||^2/2 (argmax m == argmin dist).
    ||c||^2/2 is subtracted by DVE during the PSUM->SBUF move.
  - argmin via DVE max8 + max_index (top-1 = first occurrence, matching
    jnp.argmin tie-break).
  - z_q = U[code] with U = codebook @ W_up.T precomputed on host in
    f64->f32 (tiny: 0.27 GFLOP); gathered row-wise from DRAM by
    indirect DMA (2KB/row) straight into the z_q output tiles.
  - losses: sum_t ||zd_t - c_code||^2 = sum(zd^2) - 2*sum_t max_m[t];
    per-partition partials are shipped out and reduced on host.
"""

import numpy as np
import ml_dtypes
from contextlib import ExitStack

import concourse.bass as bass
import concourse.mybir as mybir
import concourse.tile as tile
from concourse import bacc
from concourse.bass_utils import run_bass_kernel_spmd

BF = ml_dtypes.bfloat16
N_CORES = 8
B, T, D_IN, D_EMB, K = 16, 4096, 512, 256, 1024
TOK = B * T                # 65536
TPC = TOK // N_CORES       # 8192 tokens per core
NBLK = TPC // 512          # 16 blocks of 512 tokens
NCH = TPC // 128           # 64 chunks of 128 tokens

F32 = mybir.dt.float32
BF16 = mybir.dt.bfloat16
U32 = mybir.dt.uint32
SQUARE = mybir.ActivationFunctionType.Square


def _split_bf16(x):
    h = x.astype(BF)
    l = (x - h.astype(np.float32)).astype(BF)
    return h, l


def _build_nc():
    nc = bacc.Bacc("TRN2", target_bir_lowering=False, debug=False)

    def din(name, shape, dt):
        return nc.dram_tensor(name, shape, dt, kind="ExternalInput").ap()

    def dout(name, shape, dt):
        return nc.dram_tensor(name, shape, dt, kind="ExternalOutput").ap()

    zh = din("zh", [D_IN, TPC], BF16)   # z_e^T hi (host-transposed)
    zl = din("zl", [D_IN, TPC], BF16)   # z_e^T lo
    wdh = din("wdh", [D_IN, D_EMB], BF16)   # W_down.T hi
    wdl = din("wdl", [D_IN, D_EMB], BF16)   # W_down.T lo
    cbh = din("cbh", [D_EMB, K], BF16)      # codebook.T hi
    cbl = din("cbl", [D_EMB, K], BF16)      # codebook.T lo
    c2h = din("c2h", [1, K], F32)           # 0.5*||c_k||^2
    U = din("U", [K, D_IN], F32)            # codebook @ W_up.T (host f64->f32)

    zdt_o = dout("zdt_o", [D_EMB, TPC], F32)     # z_e_down transposed
    zq_o = dout("zq_o", [TPC, D_IN], F32)
    idx8_o = dout("idx8_o", [128, NCH, 8], U32)
    mv8_o = dout("mv8_o", [128, NCH, 8], F32)
    zd2_o = dout("zd2_o", [128, 2 * NBLK], F32)  # per-partition sum(zd^2)

    with tile.TileContext(nc) as tc, ExitStack() as ctx:
        consts = ctx.enter_context(tc.tile_pool(name="consts", bufs=1))
        zin = ctx.enter_context(tc.tile_pool(name="zin", bufs=2))
        zdtp = ctx.enter_context(tc.tile_pool(name="zdtp", bufs=2))
        mp = ctx.enter_context(tc.tile_pool(name="mp", bufs=3))
        zqp = ctx.enter_context(tc.tile_pool(name="zqp", bufs=3))
        psA = ctx.enter_context(tc.tile_pool(name="psA", bufs=1, space="PSUM"))
        psM = ctx.enter_context(tc.tile_pool(name="psM", bufs=2, space="PSUM"))

        # --- replicated constants ---
        wdh_t = consts.tile([128, 4, D_EMB], BF16)
        wdl_t = consts.tile([128, 4, D_EMB], BF16)
        nc.gpsimd.dma_start(out=wdh_t[:], in_=wdh.rearrange("(i p) e -> p i e", p=128))
        nc.gpsimd.dma_start(out=wdl_t[:], in_=wdl.rearrange("(i p) e -> p i e", p=128))
        cbh_t = consts.tile([128, 2, K], BF16)
        cbl_t = consts.tile([128, 2, K], BF16)
        nc.gpsimd.dma_start(out=cbh_t[:], in_=cbh.rearrange("(j p) k -> p j k", p=128))
        nc.gpsimd.dma_start(out=cbl_t[:], in_=cbl.rearrange("(j p) k -> p j k", p=128))
        c2h_t = consts.tile([128, K], F32)
        c2h_b = bass.AP(tensor=c2h.tensor, offset=c2h.offset,
                        ap=[[0, 128], list(c2h.ap[-1])])
        nc.gpsimd.dma_start(out=c2h_t[:], in_=c2h_b)

        mv8_sb = consts.tile([128, NCH, 8], F32)
        idx8_sb = consts.tile([128, NCH, 8], U32)
        zd2_sb = consts.tile([128, 2 * NBLK], F32)
        sqscr = consts.tile([128, 512], F32)

        for b in range(NBLK):
            t0 = b * 512
            # z^T tiles [128 d, 4 chunks, 512 tok] — plain contiguous loads
            zth = zin.tile([128, 4, 512], BF16)
            ztl = zin.tile([128, 4, 512], BF16)
            nc.sync.dma_start(out=zth[:],
                              in_=zh[:, t0:t0 + 512].rearrange("(i p) t -> p i t", p=128))
            nc.sync.dma_start(out=ztl[:],
                              in_=zl[:, t0:t0 + 512].rearrange("(i p) t -> p i t", p=128))

            # proj down -> z_e_down^T [e, t] in PSUM (f32)
            pzdt = psA.tile([128, 2, 512], F32)
            for j in range(2):
                mms = []
                for i in range(4):
                    js = slice(128 * j, 128 * (j + 1))
                    mms.append((wdh_t[:, i, js], zth[:, i, :]))
                    mms.append((wdl_t[:, i, js], zth[:, i, :]))
                    mms.append((wdh_t[:, i, js], ztl[:, i, :]))
                for q, (lw, rv) in enumerate(mms):
                    nc.tensor.matmul(out=pzdt[:, j, :], lhsT=lw, rhs=rv,
                                     start=(q == 0), stop=(q == len(mms) - 1))

            zdtf = zdtp.tile([128, 2, 512], F32)
            zdth = zdtp.tile([128, 2, 512], BF16)
            zdtl = zdtp.tile([128, 2, 512], BF16)
            for j in range(2):
                nc.scalar.copy(out=zdtf[:, j, :], in_=pzdt[:, j, :])
                nc.scalar.copy(out=zdth[:, j, :], in_=zdtf[:, j, :])
                nc.vector.tensor_sub(zdtl[:, j, :], zdtf[:, j, :], zdth[:, j, :])
                nc.scalar.dma_start(out=zdt_o[128 * j:128 * (j + 1), t0:t0 + 512],
                                    in_=zdtf[:, j, :])
                nc.scalar.activation(out=sqscr[:], in_=zdtf[:, j, :], func=SQUARE,
                                     accum_out=zd2_sb[:, 2 * b + j:2 * b + j + 1])

            for tci in range(4):
                c = 4 * b + tci
                tsl = slice(128 * tci, 128 * (tci + 1))
                pm = psM.tile([128, 2, 512], F32)
                for h in range(2):
                    ks = slice(512 * h, 512 * (h + 1))
                    mms = []
                    for j in range(2):
                        mms.append((zdth[:, j, tsl], cbh_t[:, j, ks]))
                        mms.append((zdth[:, j, tsl], cbl_t[:, j, ks]))
                        mms.append((zdtl[:, j, tsl], cbh_t[:, j, ks]))
                    for q, (lw, rv) in enumerate(mms):
                        nc.tensor.matmul(out=pm[:, h, :], lhsT=lw, rhs=rv,
                                         start=(q == 0), stop=(q == len(mms) - 1))
                m_sb = mp.tile([128, K], F32)
                nc.vector.tensor_sub(m_sb[:],
                                     pm[:].rearrange("p a b -> p (a b)"),
                                     c2h_t[:])
                nc.vector.max(out=mv8_sb[:, c, :], in_=m_sb[:])
                nc.vector.max_index(out=idx8_sb[:, c, :], in_max=mv8_sb[:, c, :],
                                    in_values=m_sb[:])
                zq_t = zqp.tile([128, D_IN], F32)
                nc.gpsimd.indirect_dma_start(
                    out=zq_t[:], out_offset=None, in_=U[:],
                    in_offset=bass.IndirectOffsetOnAxis(ap=idx8_sb[:, c, 0:1], axis=0))
                nc.gpsimd.dma_start(out=zq_o[t0 + 128 * tci:t0 + 128 * (tci + 1), :],
                                    in_=zq_t[:])

        nc.gpsimd.dma_start(out=idx8_o[:], in_=idx8_sb[:])
        nc.gpsimd.dma_start(out=mv8_o[:], in_=mv8_sb[:])
        nc.gpsimd.dma_start(out=zd2_o[:], in_=zd2_sb[:])

    nc.compile()
    return nc


_NC_CACHE = []


def kernel(z_e, codebook, W_down, W_up, **run_kwargs):
    z_e = np.ascontiguousarray(np.asarray(z_e, dtype=np.float32)).reshape(TOK, D_IN)
    codebook = np.asarray(codebook, dtype=np.float32)
    W_down = np.asarray(W_down, dtype=np.float32)
    W_up = np.asarray(W_up, dtype=np.float32)

    # host prep: bf16 hi/lo splits + tiny precomputes
    zT = np.ascontiguousarray(z_e.T)                             # [512, 65536]
    zh, zl = _split_bf16(zT)
    wdh, wdl = _split_bf16(np.ascontiguousarray(W_down.T))       # [512, 256]
    cbh, cbl = _split_bf16(np.ascontiguousarray(codebook.T))     # [256, 1024]
    c2h = (0.5 * np.sum(codebook.astype(np.float64) ** 2, axis=1)
           ).astype(np.float32).reshape(1, K)
    U = (codebook.astype(np.float64) @ W_up.astype(np.float64).T
         ).astype(np.float32)                                    # [1024, 512]

    if not _NC_CACHE:
        _NC_CACHE.append(_build_nc())
    nc = _NC_CACHE[0]

    in_maps = []
    for ci in range(N_CORES):
        s = slice(ci * TPC, (ci + 1) * TPC)
        in_maps.append({
            "zh": np.ascontiguousarray(zh[:, s]),
            "zl": np.ascontiguousarray(zl[:, s]),
            "wdh": wdh, "wdl": wdl, "cbh": cbh, "cbl": cbl,
            "c2h": c2h, "U": U,
        })

    res = run_bass_kernel_spmd(nc, in_maps, core_ids=list(range(N_CORES)),
                               **run_kwargs)
    results = res.results

    z_q = np.empty((TOK, D_IN), dtype=np.float32)
    z_e_down = np.empty((TOK, D_EMB), dtype=np.float32)
    code = np.empty(TOK, dtype=np.int32)
    losses = np.empty(B, dtype=np.float32)
    for ci, r in enumerate(results):
        s = slice(ci * TPC, (ci + 1) * TPC)
        z_q[s] = r["zq_o"]
        z_e_down[s] = np.ascontiguousarray(r["zdt_o"].T)
        code[s] = np.ascontiguousarray(
            r["idx8_o"][:, :, 0].astype(np.int64).T).reshape(TPC).astype(np.int32)
        maxm = np.ascontiguousarray(r["mv8_o"][:, :, 0].T).reshape(TPC)
        zd2 = r["zd2_o"]
        for lb in range(2):   # 2 batches per core
            sum_mm = maxm[4096 * lb:4096 * (lb + 1)].astype(np.float64).sum()
            sum_z2 = zd2[:, 16 * lb:16 * (lb + 1)].astype(np.float64).sum()
            losses[2 * ci + lb] = np.float32(
                (sum_z2 - 2.0 * sum_mm) / (4096.0 * 256.0))

    z_q = z_q.reshape(B, T, D_IN)
    z_e_down = z_e_down.reshape(B, T, D_EMB)
    code = code.reshape(B, T)
    if run_kwargs:
        kernel.last_results = res
    return z_q, z_e_down, code, losses, losses.copy()


# revision 14
# speedup vs baseline: 1.3292x; 1.3292x over previous
"""VQ codebook forward (nn_Codebook) on 8 Trainium2 NeuronCores.

Strategy (data-parallel over tokens, weights replicated):
  - 65536 tokens sharded 8192/core (2 full batches per core, so the
    per-batch losses stay core-local).
  - All big matmuls run as bf16 hi/lo 3-term products accumulated in
    f32 PSUM (error ~1e-5 absolute, fp32-grade; validated 0 argmin
    flips vs the f32 reference on the real data).
  - proj-down produces z_e_down TRANSPOSED ([e, t]) so it can feed the
    scores matmul as the PE stationary operand with contraction e on
    partitions; the host un-transposes the [256, 8192] output.
  - scores m[t,k] = <zd,c> - ||c||^2/2 (argmax m == argmin dist).
    ||c||^2/2 is subtracted by DVE during the PSUM->SBUF move.
  - argmin via DVE max8 + max_index (top-1 = first occurrence, matching
    jnp.argmin tie-break).
  - z_q = U[code] with U = codebook @ W_up.T precomputed on host in
    f64->f32 (tiny: 0.27 GFLOP); gathered row-wise from DRAM by
    indirect DMA (2KB/row) straight into the z_q output tiles.
  - losses: sum_t ||zd_t - c_code||^2 = sum(zd^2) - 2*sum_t max_m[t];
    per-partition partials are shipped out and reduced on host.
"""

import numpy as np
import ml_dtypes
from contextlib import ExitStack

import concourse.bass as bass
import concourse.mybir as mybir
import concourse.tile as tile
from concourse import bacc
from concourse.bass_utils import run_bass_kernel_spmd

BF = ml_dtypes.bfloat16
N_CORES = 8
B, T, D_IN, D_EMB, K = 16, 4096, 512, 256, 1024
TOK = B * T                # 65536
TPC = TOK // N_CORES       # 8192 tokens per core
NBLK = TPC // 512          # 16 blocks of 512 tokens
NCH = TPC // 128           # 64 chunks of 128 tokens

F32 = mybir.dt.float32
BF16 = mybir.dt.bfloat16
U32 = mybir.dt.uint32
SQUARE = mybir.ActivationFunctionType.Square


def _split_bf16(x):
    h = x.astype(BF)
    l = (x - h.astype(np.float32)).astype(BF)
    return h, l


def _build_nc():
    nc = bacc.Bacc("TRN2", target_bir_lowering=False, debug=False)

    def din(name, shape, dt):
        return nc.dram_tensor(name, shape, dt, kind="ExternalInput").ap()

    def dout(name, shape, dt):
        return nc.dram_tensor(name, shape, dt, kind="ExternalOutput").ap()

    zh = din("zh", [D_IN, TPC], BF16)   # z_e^T hi (host-transposed)
    zl = din("zl", [D_IN, TPC], BF16)   # z_e^T lo
    wdh = din("wdh", [D_IN, D_EMB], BF16)   # W_down.T hi
    wdl = din("wdl", [D_IN, D_EMB], BF16)   # W_down.T lo
    cbh = din("cbh", [D_EMB, K], BF16)      # codebook.T hi
    cbl = din("cbl", [D_EMB, K], BF16)      # codebook.T lo
    c2h = din("c2h", [1, K], F32)           # 0.5*||c_k||^2
    U = din("U", [K, D_IN], F32)            # codebook @ W_up.T (host f64->f32)

    zdt_o = dout("zdt_o", [D_EMB, TPC], F32)     # z_e_down transposed
    zq_o = dout("zq_o", [TPC, D_IN], F32)
    idx8_o = dout("idx8_o", [128, NCH, 8], U32)
    mv8_o = dout("mv8_o", [128, NCH, 8], F32)
    zd2_o = dout("zd2_o", [128, 2 * NBLK], F32)  # per-partition sum(zd^2)

    with tile.TileContext(nc) as tc, ExitStack() as ctx:
        consts = ctx.enter_context(tc.tile_pool(name="consts", bufs=1))
        zin = ctx.enter_context(tc.tile_pool(name="zin", bufs=2))
        zdtp = ctx.enter_context(tc.tile_pool(name="zdtp", bufs=2))
        mp = ctx.enter_context(tc.tile_pool(name="mp", bufs=3))
        zqp = ctx.enter_context(tc.tile_pool(name="zqp", bufs=6))
        psA = ctx.enter_context(tc.tile_pool(name="psA", bufs=2, space="PSUM"))
        psM = ctx.enter_context(tc.tile_pool(name="psM", bufs=2, space="PSUM"))

        # --- replicated constants ---
        wdh_t = consts.tile([128, 4, D_EMB], BF16)
        wdl_t = consts.tile([128, 4, D_EMB], BF16)
        nc.gpsimd.dma_start(out=wdh_t[:], in_=wdh.rearrange("(i p) e -> p i e", p=128))
        nc.gpsimd.dma_start(out=wdl_t[:], in_=wdl.rearrange("(i p) e -> p i e", p=128))
        cbh_t = consts.tile([128, 2, K], BF16)
        cbl_t = consts.tile([128, 2, K], BF16)
        nc.gpsimd.dma_start(out=cbh_t[:], in_=cbh.rearrange("(j p) k -> p j k", p=128))
        nc.gpsimd.dma_start(out=cbl_t[:], in_=cbl.rearrange("(j p) k -> p j k", p=128))
        c2h_t = consts.tile([128, K], F32)
        c2h_b = bass.AP(tensor=c2h.tensor, offset=c2h.offset,
                        ap=[[0, 128], list(c2h.ap[-1])])
        nc.gpsimd.dma_start(out=c2h_t[:], in_=c2h_b)

        mv8_sb = consts.tile([128, NCH, 8], F32)
        idx8_sb = consts.tile([128, NCH, 8], U32)
        zd2_sb = consts.tile([128, 2 * NBLK], F32)
        sqscr = consts.tile([128, 512], F32)

        for b in range(NBLK):
            t0 = b * 512
            # z^T tiles [128 d, 4 chunks, 512 tok] — plain contiguous loads
            zth = zin.tile([128, 4, 512], BF16)
            ztl = zin.tile([128, 4, 512], BF16)
            nc.sync.dma_start(out=zth[:],
                              in_=zh[:, t0:t0 + 512].rearrange("(i p) t -> p i t", p=128))
            nc.sync.dma_start(out=ztl[:],
                              in_=zl[:, t0:t0 + 512].rearrange("(i p) t -> p i t", p=128))

            # proj down -> z_e_down^T [e, t] in PSUM (f32)
            pzdt = psA.tile([128, 2, 512], F32)
            for j in range(2):
                mms = []
                for i in range(4):
                    js = slice(128 * j, 128 * (j + 1))
                    mms.append((wdh_t[:, i, js], zth[:, i, :]))
                    mms.append((wdl_t[:, i, js], zth[:, i, :]))
                    mms.append((wdh_t[:, i, js], ztl[:, i, :]))
                for q, (lw, rv) in enumerate(mms):
                    nc.tensor.matmul(out=pzdt[:, j, :], lhsT=lw, rhs=rv,
                                     start=(q == 0), stop=(q == len(mms) - 1))

            zdtf = zdtp.tile([128, 2, 512], F32)
            zdth = zdtp.tile([128, 2, 512], BF16)
            zdtl = zdtp.tile([128, 2, 512], BF16)
            for j in range(2):
                nc.scalar.copy(out=zdtf[:, j, :], in_=pzdt[:, j, :])
                nc.scalar.copy(out=zdth[:, j, :], in_=zdtf[:, j, :])
                nc.vector.tensor_sub(zdtl[:, j, :], zdtf[:, j, :], zdth[:, j, :])
                nc.scalar.dma_start(out=zdt_o[128 * j:128 * (j + 1), t0:t0 + 512],
                                    in_=zdtf[:, j, :])
                nc.scalar.activation(out=sqscr[:], in_=zdtf[:, j, :], func=SQUARE,
                                     accum_out=zd2_sb[:, 2 * b + j:2 * b + j + 1])

            for tci in range(4):
                c = 4 * b + tci
                tsl = slice(128 * tci, 128 * (tci + 1))
                pm = psM.tile([128, 2, 512], F32)
                for h in range(2):
                    ks = slice(512 * h, 512 * (h + 1))
                    mms = []
                    for j in range(2):
                        mms.append((zdth[:, j, tsl], cbh_t[:, j, ks]))
                        mms.append((zdth[:, j, tsl], cbl_t[:, j, ks]))
                        mms.append((zdtl[:, j, tsl], cbh_t[:, j, ks]))
                    for q, (lw, rv) in enumerate(mms):
                        nc.tensor.matmul(out=pm[:, h, :], lhsT=lw, rhs=rv,
                                         start=(q == 0), stop=(q == len(mms) - 1))
                m_sb = mp.tile([128, K], F32)
                nc.vector.tensor_sub(m_sb[:],
                                     pm[:].rearrange("p a b -> p (a b)"),
                                     c2h_t[:])
                nc.vector.max(out=mv8_sb[:, c, :], in_=m_sb[:])
                nc.vector.max_index(out=idx8_sb[:, c, :], in_max=mv8_sb[:, c, :],
                                    in_values=m_sb[:])
                zq_t = zqp.tile([128, D_IN], F32)
                nc.gpsimd.indirect_dma_start(
                    out=zq_t[:], out_offset=None, in_=U[:],
                    in_offset=bass.IndirectOffsetOnAxis(ap=idx8_sb[:, c, 0:1], axis=0))
                nc.sync.dma_start(out=zq_o[t0 + 128 * tci:t0 + 128 * (tci + 1), :],
                                  in_=zq_t[:])

        nc.sync.dma_start(out=idx8_o[:], in_=idx8_sb[:])
        nc.scalar.dma_start(out=mv8_o[:], in_=mv8_sb[:])
        nc.scalar.dma_start(out=zd2_o[:], in_=zd2_sb[:])

    nc.compile()
    return nc


_NC_CACHE = []


def kernel(z_e, codebook, W_down, W_up, **run_kwargs):
    z_e = np.ascontiguousarray(np.asarray(z_e, dtype=np.float32)).reshape(TOK, D_IN)
    codebook = np.asarray(codebook, dtype=np.float32)
    W_down = np.asarray(W_down, dtype=np.float32)
    W_up = np.asarray(W_up, dtype=np.float32)

    # host prep: bf16 hi/lo splits + tiny precomputes
    zT = np.ascontiguousarray(z_e.T)                             # [512, 65536]
    zh, zl = _split_bf16(zT)
    wdh, wdl = _split_bf16(np.ascontiguousarray(W_down.T))       # [512, 256]
    cbh, cbl = _split_bf16(np.ascontiguousarray(codebook.T))     # [256, 1024]
    c2h = (0.5 * np.sum(codebook.astype(np.float64) ** 2, axis=1)
           ).astype(np.float32).reshape(1, K)
    U = (codebook.astype(np.float64) @ W_up.astype(np.float64).T
         ).astype(np.float32)                                    # [1024, 512]

    if not _NC_CACHE:
        _NC_CACHE.append(_build_nc())
    nc = _NC_CACHE[0]

    in_maps = []
    for ci in range(N_CORES):
        s = slice(ci * TPC, (ci + 1) * TPC)
        in_maps.append({
            "zh": np.ascontiguousarray(zh[:, s]),
            "zl": np.ascontiguousarray(zl[:, s]),
            "wdh": wdh, "wdl": wdl, "cbh": cbh, "cbl": cbl,
            "c2h": c2h, "U": U,
        })

    res = run_bass_kernel_spmd(nc, in_maps, core_ids=list(range(N_CORES)),
                               **run_kwargs)
    results = res.results

    z_q = np.empty((TOK, D_IN), dtype=np.float32)
    z_e_down = np.empty((TOK, D_EMB), dtype=np.float32)
    code = np.empty(TOK, dtype=np.int32)
    losses = np.empty(B, dtype=np.float32)
    for ci, r in enumerate(results):
        s = slice(ci * TPC, (ci + 1) * TPC)
        z_q[s] = r["zq_o"]
        z_e_down[s] = np.ascontiguousarray(r["zdt_o"].T)
        code[s] = np.ascontiguousarray(
            r["idx8_o"][:, :, 0].astype(np.int64).T).reshape(TPC).astype(np.int32)
        maxm = np.ascontiguousarray(r["mv8_o"][:, :, 0].T).reshape(TPC)
        zd2 = r["zd2_o"]
        for lb in range(2):   # 2 batches per core
            sum_mm = maxm[4096 * lb:4096 * (lb + 1)].astype(np.float64).sum()
            sum_z2 = zd2[:, 16 * lb:16 * (lb + 1)].astype(np.float64).sum()
            losses[2 * ci + lb] = np.float32(
                (sum_z2 - 2.0 * sum_mm) / (4096.0 * 256.0))

    z_q = z_q.reshape(B, T, D_IN)
    z_e_down = z_e_down.reshape(B, T, D_EMB)
    code = code.reshape(B, T)
    if run_kwargs:
        kernel.last_results = res
    return z_q, z_e_down, code, losses, losses.copy()


# revision 17
# speedup vs baseline: 1.4288x; 1.0749x over previous
"""VQ codebook forward (nn_Codebook) on 8 Trainium2 NeuronCores.

Strategy (data-parallel over tokens, weights replicated):
  - 65536 tokens sharded 8192/core (2 full batches per core, so the
    per-batch losses stay core-local).
  - All big matmuls run as bf16 hi/lo 3-term products accumulated in
    f32 PSUM (error ~1e-5 absolute, fp32-grade; validated 0 argmin
    flips vs the f32 reference on the real data).
  - proj-down produces z_e_down TRANSPOSED ([e, t]) so it can feed the
    scores matmul as the PE stationary operand with contraction e on
    partitions; the host un-transposes the [256, 8192] output.
  - scores m[t,k] = <zd,c> - ||c||^2/2 (argmax m == argmin dist).
    ||c||^2/2 is subtracted by DVE during the PSUM->SBUF move.
  - argmin via DVE max8 + max_index (top-1 = first occurrence, matching
    jnp.argmin tie-break).
  - z_q = U[code] with U = codebook @ W_up.T precomputed on host in
    f64->f32 (tiny: 0.27 GFLOP); gathered row-wise from DRAM by
    indirect DMA (2KB/row) straight into the z_q output tiles.
  - losses: sum_t ||zd_t - c_code||^2 = sum(zd^2) - 2*sum_t max_m[t];
    per-partition partials are shipped out and reduced on host.
"""

import numpy as np
import ml_dtypes
from contextlib import ExitStack

import concourse.bass as bass
import concourse.mybir as mybir
import concourse.tile as tile
from concourse import bacc
from concourse.bass_utils import run_bass_kernel_spmd

BF = ml_dtypes.bfloat16
N_CORES = 8
B, T, D_IN, D_EMB, K = 16, 4096, 512, 256, 1024
TOK = B * T                # 65536
TPC = TOK // N_CORES       # 8192 tokens per core
NBLK = TPC // 512          # 16 blocks of 512 tokens
NCH = TPC // 128           # 64 chunks of 128 tokens

F32 = mybir.dt.float32
BF16 = mybir.dt.bfloat16
U32 = mybir.dt.uint32
SQUARE = mybir.ActivationFunctionType.Square


def _split_bf16(x):
    h = x.astype(BF)
    l = (x - h.astype(np.float32)).astype(BF)
    return h, l


def _build_nc():
    nc = bacc.Bacc("TRN2", target_bir_lowering=False, debug=False)

    def din(name, shape, dt):
        return nc.dram_tensor(name, shape, dt, kind="ExternalInput").ap()

    def dout(name, shape, dt):
        return nc.dram_tensor(name, shape, dt, kind="ExternalOutput").ap()

    zh = din("zh", [D_IN, TPC], BF16)   # z_e^T hi (host-transposed)
    zl = din("zl", [D_IN, TPC], BF16)   # z_e^T lo
    wdh = din("wdh", [D_IN, D_EMB], BF16)   # W_down.T hi
    wdl = din("wdl", [D_IN, D_EMB], BF16)   # W_down.T lo
    cbh = din("cbh", [D_EMB, K], BF16)      # codebook.T hi
    cbl = din("cbl", [D_EMB, K], BF16)      # codebook.T lo
    c2h = din("c2h", [1, K], F32)           # 0.5*||c_k||^2
    U = din("U", [K, D_IN], F32)            # codebook @ W_up.T (host f64->f32)

    zdt_o = dout("zdt_o", [D_EMB, TPC], F32)     # z_e_down transposed
    zq_o = dout("zq_o", [TPC, D_IN], F32)
    idx8_o = dout("idx8_o", [128, NCH, 8], U32)
    mv8_o = dout("mv8_o", [128, NCH, 8], F32)
    zd2_o = dout("zd2_o", [128, 2 * NBLK], F32)  # per-partition sum(zd^2)

    with tile.TileContext(nc) as tc, ExitStack() as ctx:
        consts = ctx.enter_context(tc.tile_pool(name="consts", bufs=1))
        zin = ctx.enter_context(tc.tile_pool(name="zin", bufs=2))
        zdtp = ctx.enter_context(tc.tile_pool(name="zdtp", bufs=2))
        mp = ctx.enter_context(tc.tile_pool(name="mp", bufs=3))
        zqp = ctx.enter_context(tc.tile_pool(name="zqp", bufs=6))
        psA = ctx.enter_context(tc.tile_pool(name="psA", bufs=3, space="PSUM"))
        psM = ctx.enter_context(tc.tile_pool(name="psM", bufs=4, space="PSUM"))

        # --- replicated constants ---
        wdh_t = consts.tile([128, 4, D_EMB], BF16)
        wdl_t = consts.tile([128, 4, D_EMB], BF16)
        nc.gpsimd.dma_start(out=wdh_t[:], in_=wdh.rearrange("(i p) e -> p i e", p=128))
        nc.gpsimd.dma_start(out=wdl_t[:], in_=wdl.rearrange("(i p) e -> p i e", p=128))
        cbh_t = consts.tile([128, 2, K], BF16)
        cbl_t = consts.tile([128, 2, K], BF16)
        nc.gpsimd.dma_start(out=cbh_t[:], in_=cbh.rearrange("(j p) k -> p j k", p=128))
        nc.gpsimd.dma_start(out=cbl_t[:], in_=cbl.rearrange("(j p) k -> p j k", p=128))
        c2h_t = consts.tile([128, K], F32)
        c2h_b = bass.AP(tensor=c2h.tensor, offset=c2h.offset,
                        ap=[[0, 128], list(c2h.ap[-1])])
        nc.gpsimd.dma_start(out=c2h_t[:], in_=c2h_b)

        mv8_sb = consts.tile([128, NCH, 8], F32)
        idx8_sb = consts.tile([128, NCH, 8], U32)
        zd2_sb = consts.tile([128, 2 * NBLK], F32)
        sqscr = consts.tile([128, 512], F32)

        for b in range(NBLK):
            t0 = b * 512
            # z^T tiles [128 d, 4 chunks, 512 tok] — plain contiguous loads
            zth = zin.tile([128, 4, 512], BF16)
            ztl = zin.tile([128, 4, 512], BF16)
            nc.sync.dma_start(out=zth[:],
                              in_=zh[:, t0:t0 + 512].rearrange("(i p) t -> p i t", p=128))
            nc.sync.dma_start(out=ztl[:],
                              in_=zl[:, t0:t0 + 512].rearrange("(i p) t -> p i t", p=128))

            # proj down -> z_e_down^T [e, t] in PSUM (f32), one bank per e-chunk
            zdtf = zdtp.tile([128, 2, 512], F32)
            zdth = zdtp.tile([128, 2, 512], BF16)
            zdtl = zdtp.tile([128, 2, 512], BF16)
            for j in range(2):
                pzdt = psA.tile([128, 512], F32)
                mms = []
                for i in range(4):
                    js = slice(128 * j, 128 * (j + 1))
                    mms.append((wdh_t[:, i, js], zth[:, i, :]))
                    mms.append((wdl_t[:, i, js], zth[:, i, :]))
                    mms.append((wdh_t[:, i, js], ztl[:, i, :]))
                for q, (lw, rv) in enumerate(mms):
                    nc.tensor.matmul(out=pzdt[:], lhsT=lw, rhs=rv,
                                     start=(q == 0), stop=(q == len(mms) - 1))
                nc.scalar.copy(out=zdtf[:, j, :], in_=pzdt[:])
                nc.scalar.copy(out=zdth[:, j, :], in_=zdtf[:, j, :])
                nc.vector.tensor_sub(zdtl[:, j, :], zdtf[:, j, :], zdth[:, j, :])
                nc.scalar.dma_start(out=zdt_o[128 * j:128 * (j + 1), t0:t0 + 512],
                                    in_=zdtf[:, j, :])
                nc.scalar.activation(out=sqscr[:], in_=zdtf[:, j, :], func=SQUARE,
                                     accum_out=zd2_sb[:, 2 * b + j:2 * b + j + 1])

            for tci in range(4):
                c = 4 * b + tci
                tsl = slice(128 * tci, 128 * (tci + 1))
                m_sb = mp.tile([128, K], F32)
                for h in range(2):
                    pm = psM.tile([128, 512], F32)
                    ks = slice(512 * h, 512 * (h + 1))
                    mms = []
                    for j in range(2):
                        mms.append((zdth[:, j, tsl], cbh_t[:, j, ks]))
                        mms.append((zdth[:, j, tsl], cbl_t[:, j, ks]))
                        mms.append((zdtl[:, j, tsl], cbh_t[:, j, ks]))
                    for q, (lw, rv) in enumerate(mms):
                        nc.tensor.matmul(out=pm[:], lhsT=lw, rhs=rv,
                                         start=(q == 0), stop=(q == len(mms) - 1))
                    nc.vector.tensor_sub(m_sb[:, ks], pm[:], c2h_t[:, ks])
                nc.vector.max(out=mv8_sb[:, c, :], in_=m_sb[:])
                nc.vector.max_index(out=idx8_sb[:, c, :], in_max=mv8_sb[:, c, :],
                                    in_values=m_sb[:])
                zq_t = zqp.tile([128, D_IN], F32)
                nc.gpsimd.indirect_dma_start(
                    out=zq_t[:], out_offset=None, in_=U[:],
                    in_offset=bass.IndirectOffsetOnAxis(ap=idx8_sb[:, c, 0:1], axis=0))
                nc.sync.dma_start(out=zq_o[t0 + 128 * tci:t0 + 128 * (tci + 1), :],
                                  in_=zq_t[:])

        nc.sync.dma_start(out=idx8_o[:], in_=idx8_sb[:])
        nc.scalar.dma_start(out=mv8_o[:], in_=mv8_sb[:])
        nc.scalar.dma_start(out=zd2_o[:], in_=zd2_sb[:])

    nc.compile()
    return nc


_NC_CACHE = []


def kernel(z_e, codebook, W_down, W_up, **run_kwargs):
    z_e = np.ascontiguousarray(np.asarray(z_e, dtype=np.float32)).reshape(TOK, D_IN)
    codebook = np.asarray(codebook, dtype=np.float32)
    W_down = np.asarray(W_down, dtype=np.float32)
    W_up = np.asarray(W_up, dtype=np.float32)

    # host prep: bf16 hi/lo splits + tiny precomputes
    zT = np.ascontiguousarray(z_e.T)                             # [512, 65536]
    zh, zl = _split_bf16(zT)
    wdh, wdl = _split_bf16(np.ascontiguousarray(W_down.T))       # [512, 256]
    cbh, cbl = _split_bf16(np.ascontiguousarray(codebook.T))     # [256, 1024]
    c2h = (0.5 * np.sum(codebook.astype(np.float64) ** 2, axis=1)
           ).astype(np.float32).reshape(1, K)
    U = (codebook.astype(np.float64) @ W_up.astype(np.float64).T
         ).astype(np.float32)                                    # [1024, 512]

    if not _NC_CACHE:
        _NC_CACHE.append(_build_nc())
    nc = _NC_CACHE[0]

    in_maps = []
    for ci in range(N_CORES):
        s = slice(ci * TPC, (ci + 1) * TPC)
        in_maps.append({
            "zh": np.ascontiguousarray(zh[:, s]),
            "zl": np.ascontiguousarray(zl[:, s]),
            "wdh": wdh, "wdl": wdl, "cbh": cbh, "cbl": cbl,
            "c2h": c2h, "U": U,
        })

    res = run_bass_kernel_spmd(nc, in_maps, core_ids=list(range(N_CORES)),
                               **run_kwargs)
    results = res.results

    z_q = np.empty((TOK, D_IN), dtype=np.float32)
    z_e_down = np.empty((TOK, D_EMB), dtype=np.float32)
    code = np.empty(TOK, dtype=np.int32)
    losses = np.empty(B, dtype=np.float32)
    for ci, r in enumerate(results):
        s = slice(ci * TPC, (ci + 1) * TPC)
        z_q[s] = r["zq_o"]
        z_e_down[s] = np.ascontiguousarray(r["zdt_o"].T)
        code[s] = np.ascontiguousarray(
            r["idx8_o"][:, :, 0].astype(np.int64).T).reshape(TPC).astype(np.int32)
        maxm = np.ascontiguousarray(r["mv8_o"][:, :, 0].T).reshape(TPC)
        zd2 = r["zd2_o"]
        for lb in range(2):   # 2 batches per core
            sum_mm = maxm[4096 * lb:4096 * (lb + 1)].astype(np.float64).sum()
            sum_z2 = zd2[:, 16 * lb:16 * (lb + 1)].astype(np.float64).sum()
            losses[2 * ci + lb] = np.float32(
                (sum_z2 - 2.0 * sum_mm) / (4096.0 * 256.0))

    z_q = z_q.reshape(B, T, D_IN)
    z_e_down = z_e_down.reshape(B, T, D_EMB)
    code = code.reshape(B, T)
    if run_kwargs:
        kernel.last_results = res
    return z_q, z_e_down, code, losses, losses.copy()


# revision 20
# speedup vs baseline: 1.4357x; 1.0049x over previous
"""VQ codebook forward (nn_Codebook) on 8 Trainium2 NeuronCores.

Strategy (data-parallel over tokens, weights replicated):
  - 65536 tokens sharded 8192/core (2 full batches per core, so the
    per-batch losses stay core-local).
  - All big matmuls run as bf16 hi/lo 3-term products accumulated in
    f32 PSUM (error ~1e-5 absolute, fp32-grade; validated 0 argmin
    flips vs the f32 reference on the real data).
  - proj-down produces z_e_down TRANSPOSED ([e, t]) so it can feed the
    scores matmul as the PE stationary operand with contraction e on
    partitions; the host un-transposes the [256, 8192] output.
  - scores m[t,k] = <zd,c> - ||c||^2/2 (argmax m == argmin dist).
    ||c||^2/2 is subtracted by DVE during the PSUM->SBUF move.
  - argmin via DVE max8 + max_index (top-1 = first occurrence, matching
    jnp.argmin tie-break).
  - z_q = U[code] with U = codebook @ W_up.T precomputed on host in
    f64->f32 (tiny: 0.27 GFLOP); gathered row-wise from DRAM by
    indirect DMA (2KB/row) straight into the z_q output tiles.
  - losses: sum_t ||zd_t - c_code||^2 = sum(zd^2) - 2*sum_t max_m[t];
    per-partition partials are shipped out and reduced on host.
"""

import numpy as np
import ml_dtypes
from contextlib import ExitStack

import concourse.bass as bass
import concourse.mybir as mybir
import concourse.tile as tile
from concourse import bacc
from concourse.bass_utils import run_bass_kernel_spmd

BF = ml_dtypes.bfloat16
N_CORES = 8
B, T, D_IN, D_EMB, K = 16, 4096, 512, 256, 1024
TOK = B * T                # 65536
TPC = TOK // N_CORES       # 8192 tokens per core
NBLK = TPC // 512          # 16 blocks of 512 tokens
NCH = TPC // 128           # 64 chunks of 128 tokens

F32 = mybir.dt.float32
BF16 = mybir.dt.bfloat16
U32 = mybir.dt.uint32
SQUARE = mybir.ActivationFunctionType.Square


def _split_bf16(x):
    h = x.astype(BF)
    l = (x - h.astype(np.float32)).astype(BF)
    return h, l


def _build_nc():
    nc = bacc.Bacc("TRN2", target_bir_lowering=False, debug=False)

    def din(name, shape, dt):
        return nc.dram_tensor(name, shape, dt, kind="ExternalInput").ap()

    def dout(name, shape, dt):
        return nc.dram_tensor(name, shape, dt, kind="ExternalOutput").ap()

    zh = din("zh", [D_IN, TPC], BF16)   # z_e^T hi (host-transposed)
    zl = din("zl", [D_IN, TPC], BF16)   # z_e^T lo
    wdh = din("wdh", [D_IN, D_EMB], BF16)   # W_down.T hi
    wdl = din("wdl", [D_IN, D_EMB], BF16)   # W_down.T lo
    cbh = din("cbh", [D_EMB, K], BF16)      # codebook.T hi
    cbl = din("cbl", [D_EMB, K], BF16)      # codebook.T lo
    c2h = din("c2h", [1, K], F32)           # 0.5*||c_k||^2
    U = din("U", [K, D_IN], F32)            # codebook @ W_up.T (host f64->f32)

    zdt_o = dout("zdt_o", [D_EMB, TPC], F32)     # z_e_down transposed
    zq_o = dout("zq_o", [TPC, D_IN], F32)
    idx8_o = dout("idx8_o", [128, NCH, 8], U32)
    mv8_o = dout("mv8_o", [128, NCH, 8], F32)
    zd2_o = dout("zd2_o", [128, 2 * NBLK], F32)  # per-partition sum(zd^2)

    with tile.TileContext(nc) as tc, ExitStack() as ctx:
        consts = ctx.enter_context(tc.tile_pool(name="consts", bufs=1))
        zin = ctx.enter_context(tc.tile_pool(name="zin", bufs=3))
        zdtp = ctx.enter_context(tc.tile_pool(name="zdtp", bufs=3))
        mp = ctx.enter_context(tc.tile_pool(name="mp", bufs=3))
        zqp = ctx.enter_context(tc.tile_pool(name="zqp", bufs=6))
        psA = ctx.enter_context(tc.tile_pool(name="psA", bufs=3, space="PSUM"))
        psM = ctx.enter_context(tc.tile_pool(name="psM", bufs=4, space="PSUM"))

        # --- replicated constants ---
        wdh_t = consts.tile([128, 4, D_EMB], BF16)
        wdl_t = consts.tile([128, 4, D_EMB], BF16)
        nc.gpsimd.dma_start(out=wdh_t[:], in_=wdh.rearrange("(i p) e -> p i e", p=128))
        nc.gpsimd.dma_start(out=wdl_t[:], in_=wdl.rearrange("(i p) e -> p i e", p=128))
        cbh_t = consts.tile([128, 2, K], BF16)
        cbl_t = consts.tile([128, 2, K], BF16)
        c2h_t = consts.tile([128, K], F32)

        mv8_sb = consts.tile([128, NCH, 8], F32)
        idx8_sb = consts.tile([128, NCH, 8], U32)
        zd2_sb = consts.tile([128, 2 * NBLK], F32)
        sqscr = consts.tile([128, 512], F32)

        for b in range(NBLK):
            t0 = b * 512
            # z^T tiles [128 d, 4 chunks, 512 tok] — plain contiguous loads
            zth = zin.tile([128, 4, 512], BF16)
            ztl = zin.tile([128, 4, 512], BF16)
            nc.sync.dma_start(out=zth[:],
                              in_=zh[:, t0:t0 + 512].rearrange("(i p) t -> p i t", p=128))
            nc.sync.dma_start(out=ztl[:],
                              in_=zl[:, t0:t0 + 512].rearrange("(i p) t -> p i t", p=128))

            # proj down -> z_e_down^T [e, t] in PSUM (f32), one bank per e-chunk
            zdtf = zdtp.tile([128, 2, 512], F32)
            zdth = zdtp.tile([128, 2, 512], BF16)
            zdtl = zdtp.tile([128, 2, 512], BF16)
            for j in range(2):
                pzdt = psA.tile([128, 512], F32)
                mms = []
                for i in range(4):
                    js = slice(128 * j, 128 * (j + 1))
                    mms.append((wdh_t[:, i, js], zth[:, i, :]))
                    mms.append((wdl_t[:, i, js], zth[:, i, :]))
                    mms.append((wdh_t[:, i, js], ztl[:, i, :]))
                for q, (lw, rv) in enumerate(mms):
                    nc.tensor.matmul(out=pzdt[:], lhsT=lw, rhs=rv,
                                     start=(q == 0), stop=(q == len(mms) - 1))
                nc.scalar.copy(out=zdtf[:, j, :], in_=pzdt[:])
                nc.scalar.copy(out=zdth[:, j, :], in_=zdtf[:, j, :])
                nc.vector.tensor_sub(zdtl[:, j, :], zdtf[:, j, :], zdth[:, j, :])
                nc.scalar.dma_start(out=zdt_o[128 * j:128 * (j + 1), t0:t0 + 512],
                                    in_=zdtf[:, j, :])
                nc.scalar.activation(out=sqscr[:], in_=zdtf[:, j, :], func=SQUARE,
                                     accum_out=zd2_sb[:, 2 * b + j:2 * b + j + 1])

            if b == 0:
                # codebook constants land while block-0 proj runs
                nc.gpsimd.dma_start(out=cbh_t[:],
                                    in_=cbh.rearrange("(j p) k -> p j k", p=128))
                nc.gpsimd.dma_start(out=cbl_t[:],
                                    in_=cbl.rearrange("(j p) k -> p j k", p=128))
                c2h_b = bass.AP(tensor=c2h.tensor, offset=c2h.offset,
                                ap=[[0, 128], list(c2h.ap[-1])])
                nc.gpsimd.dma_start(out=c2h_t[:], in_=c2h_b)

            for tci in range(4):
                c = 4 * b + tci
                tsl = slice(128 * tci, 128 * (tci + 1))
                m_sb = mp.tile([128, K], F32)
                for h in range(2):
                    pm = psM.tile([128, 512], F32)
                    ks = slice(512 * h, 512 * (h + 1))
                    mms = []
                    for j in range(2):
                        mms.append((zdth[:, j, tsl], cbh_t[:, j, ks]))
                        mms.append((zdth[:, j, tsl], cbl_t[:, j, ks]))
                        mms.append((zdtl[:, j, tsl], cbh_t[:, j, ks]))
                    for q, (lw, rv) in enumerate(mms):
                        nc.tensor.matmul(out=pm[:], lhsT=lw, rhs=rv,
                                         start=(q == 0), stop=(q == len(mms) - 1))
                    nc.vector.tensor_sub(m_sb[:, ks], pm[:], c2h_t[:, ks])
                nc.vector.max(out=mv8_sb[:, c, :], in_=m_sb[:])
                nc.vector.max_index(out=idx8_sb[:, c, :], in_max=mv8_sb[:, c, :],
                                    in_values=m_sb[:])
                zq_t = zqp.tile([128, D_IN], F32)
                nc.gpsimd.indirect_dma_start(
                    out=zq_t[:], out_offset=None, in_=U[:],
                    in_offset=bass.IndirectOffsetOnAxis(ap=idx8_sb[:, c, 0:1], axis=0))
                nc.sync.dma_start(out=zq_o[t0 + 128 * tci:t0 + 128 * (tci + 1), :],
                                  in_=zq_t[:])

        nc.sync.dma_start(out=idx8_o[:], in_=idx8_sb[:])
        nc.scalar.dma_start(out=mv8_o[:], in_=mv8_sb[:])
        nc.scalar.dma_start(out=zd2_o[:], in_=zd2_sb[:])

    nc.compile()
    return nc


_NC_CACHE = []


def kernel(z_e, codebook, W_down, W_up, **run_kwargs):
    z_e = np.ascontiguousarray(np.asarray(z_e, dtype=np.float32)).reshape(TOK, D_IN)
    codebook = np.asarray(codebook, dtype=np.float32)
    W_down = np.asarray(W_down, dtype=np.float32)
    W_up = np.asarray(W_up, dtype=np.float32)

    # host prep: bf16 hi/lo splits + tiny precomputes
    zT = np.ascontiguousarray(z_e.T)                             # [512, 65536]
    zh, zl = _split_bf16(zT)
    wdh, wdl = _split_bf16(np.ascontiguousarray(W_down.T))       # [512, 256]
    cbh, cbl = _split_bf16(np.ascontiguousarray(codebook.T))     # [256, 1024]
    c2h = (0.5 * np.sum(codebook.astype(np.float64) ** 2, axis=1)
           ).astype(np.float32).reshape(1, K)
    U = (codebook.astype(np.float64) @ W_up.astype(np.float64).T
         ).astype(np.float32)                                    # [1024, 512]

    if not _NC_CACHE:
        _NC_CACHE.append(_build_nc())
    nc = _NC_CACHE[0]

    in_maps = []
    for ci in range(N_CORES):
        s = slice(ci * TPC, (ci + 1) * TPC)
        in_maps.append({
            "zh": np.ascontiguousarray(zh[:, s]),
            "zl": np.ascontiguousarray(zl[:, s]),
            "wdh": wdh, "wdl": wdl, "cbh": cbh, "cbl": cbl,
            "c2h": c2h, "U": U,
        })

    res = run_bass_kernel_spmd(nc, in_maps, core_ids=list(range(N_CORES)),
                               **run_kwargs)
    results = res.results

    z_q = np.empty((TOK, D_IN), dtype=np.float32)
    z_e_down = np.empty((TOK, D_EMB), dtype=np.float32)
    code = np.empty(TOK, dtype=np.int32)
    losses = np.empty(B, dtype=np.float32)
    for ci, r in enumerate(results):
        s = slice(ci * TPC, (ci + 1) * TPC)
        z_q[s] = r["zq_o"]
        z_e_down[s] = np.ascontiguousarray(r["zdt_o"].T)
        code[s] = np.ascontiguousarray(
            r["idx8_o"][:, :, 0].astype(np.int64).T).reshape(TPC).astype(np.int32)
        maxm = np.ascontiguousarray(r["mv8_o"][:, :, 0].T).reshape(TPC)
        zd2 = r["zd2_o"]
        for lb in range(2):   # 2 batches per core
            sum_mm = maxm[4096 * lb:4096 * (lb + 1)].astype(np.float64).sum()
            sum_z2 = zd2[:, 16 * lb:16 * (lb + 1)].astype(np.float64).sum()
            losses[2 * ci + lb] = np.float32(
                (sum_z2 - 2.0 * sum_mm) / (4096.0 * 256.0))

    z_q = z_q.reshape(B, T, D_IN)
    z_e_down = z_e_down.reshape(B, T, D_EMB)
    code = code.reshape(B, T)
    if run_kwargs:
        kernel.last_results = res
    return z_q, z_e_down, code, losses, losses.copy()


# revision 23
# speedup vs baseline: 1.4501x; 1.0100x over previous
"""VQ codebook forward (nn_Codebook) on 8 Trainium2 NeuronCores.

Strategy (data-parallel over tokens, weights replicated):
  - 65536 tokens sharded 8192/core (2 full batches per core, so the
    per-batch losses stay core-local).
  - All big matmuls run as bf16 hi/lo 3-term products accumulated in
    f32 PSUM (error ~1e-5 absolute, fp32-grade; validated 0 argmin
    flips vs the f32 reference on the real data).
  - proj-down produces z_e_down TRANSPOSED ([e, t]) so it can feed the
    scores matmul as the PE stationary operand with contraction e on
    partitions; the host un-transposes the [256, 8192] output.
  - scores m[t,k] = <zd,c> - ||c||^2/2 (argmax m == argmin dist).
    ||c||^2/2 is subtracted by DVE during the PSUM->SBUF move.
  - argmin via DVE max8 + max_index (top-1 = first occurrence, matching
    jnp.argmin tie-break).
  - z_q = U[code] with U = codebook @ W_up.T precomputed on host in
    f64->f32 (tiny: 0.27 GFLOP); gathered row-wise from DRAM by
    indirect DMA (2KB/row) straight into the z_q output tiles.
  - losses: sum_t ||zd_t - c_code||^2 = sum(zd^2) - 2*sum_t max_m[t];
    per-partition partials are shipped out and reduced on host.
"""

import numpy as np
import ml_dtypes
from contextlib import ExitStack

import concourse.bass as bass
import concourse.mybir as mybir
import concourse.tile as tile
from concourse import bacc
from concourse.bass_utils import run_bass_kernel_spmd

BF = ml_dtypes.bfloat16
N_CORES = 8
B, T, D_IN, D_EMB, K = 16, 4096, 512, 256, 1024
TOK = B * T                # 65536
TPC = TOK // N_CORES       # 8192 tokens per core
NBLK = TPC // 512          # 16 blocks of 512 tokens
NCH = TPC // 128           # 64 chunks of 128 tokens

F32 = mybir.dt.float32
BF16 = mybir.dt.bfloat16
U32 = mybir.dt.uint32
SQUARE = mybir.ActivationFunctionType.Square


def _split_bf16(x):
    h = x.astype(BF)
    l = (x - h.astype(np.float32)).astype(BF)
    return h, l


def _build_nc():
    nc = bacc.Bacc("TRN2", target_bir_lowering=False, debug=False)

    def din(name, shape, dt):
        return nc.dram_tensor(name, shape, dt, kind="ExternalInput").ap()

    def dout(name, shape, dt):
        return nc.dram_tensor(name, shape, dt, kind="ExternalOutput").ap()

    zh = din("zh", [D_IN, TPC], BF16)   # z_e^T hi (host-transposed)
    zl = din("zl", [D_IN, TPC], BF16)   # z_e^T lo
    wdh = din("wdh", [D_IN, D_EMB], BF16)   # W_down.T hi
    wdl = din("wdl", [D_IN, D_EMB], BF16)   # W_down.T lo
    cbh = din("cbh", [D_EMB, K], BF16)      # codebook.T hi
    cbl = din("cbl", [D_EMB, K], BF16)      # codebook.T lo
    c2h = din("c2h", [1, K], F32)           # 0.5*||c_k||^2
    U = din("U", [K, D_IN], F32)            # codebook @ W_up.T (host f64->f32)

    zdt_o = dout("zdt_o", [D_EMB, TPC], F32)     # z_e_down transposed
    zq_o = dout("zq_o", [TPC, D_IN], F32)
    idx8_o = dout("idx8_o", [128, NCH, 8], U32)
    mv8_o = dout("mv8_o", [128, NCH, 8], F32)
    zd2_o = dout("zd2_o", [128, 2 * NBLK], F32)  # per-partition sum(zd^2)

    with tile.TileContext(nc) as tc, ExitStack() as ctx:
        consts = ctx.enter_context(tc.tile_pool(name="consts", bufs=1))
        zin = ctx.enter_context(tc.tile_pool(name="zin", bufs=3))
        zdtp = ctx.enter_context(tc.tile_pool(name="zdtp", bufs=3))
        mp = ctx.enter_context(tc.tile_pool(name="mp", bufs=3))
        zqp = ctx.enter_context(tc.tile_pool(name="zqp", bufs=6))
        psA = ctx.enter_context(tc.tile_pool(name="psA", bufs=4, space="PSUM"))
        psM = ctx.enter_context(tc.tile_pool(name="psM", bufs=2, space="PSUM"))

        # --- replicated constants ---
        wdh_t = consts.tile([128, 4, D_EMB], BF16)
        wdl_t = consts.tile([128, 4, D_EMB], BF16)
        nc.gpsimd.dma_start(out=wdh_t[:], in_=wdh.rearrange("(i p) e -> p i e", p=128))
        nc.gpsimd.dma_start(out=wdl_t[:], in_=wdl.rearrange("(i p) e -> p i e", p=128))
        cbh_t = consts.tile([128, 2, K], BF16)
        cbl_t = consts.tile([128, 2, K], BF16)
        c2h_t = consts.tile([128, K], F32)

        mv8_sb = consts.tile([128, NCH, 8], F32)
        idx8_sb = consts.tile([128, NCH, 8], U32)
        zd2_sb = consts.tile([128, 2 * NBLK], F32)
        sqscr = consts.tile([128, 512], F32)

        for b in range(NBLK):
            t0 = b * 512
            # z^T tiles [128 d, 4 chunks, 512 tok] — plain contiguous loads
            zth = zin.tile([128, 4, 512], BF16)
            ztl = zin.tile([128, 4, 512], BF16)
            nc.sync.dma_start(out=zth[:],
                              in_=zh[:, t0:t0 + 512].rearrange("(i p) t -> p i t", p=128))
            nc.sync.dma_start(out=ztl[:],
                              in_=zl[:, t0:t0 + 512].rearrange("(i p) t -> p i t", p=128))

            # proj down -> z_e_down^T [e, t] in PSUM (f32), one bank per e-chunk
            zdtf = zdtp.tile([128, 2, 512], F32)
            zdth = zdtp.tile([128, 2, 512], BF16)
            zdtl = zdtp.tile([128, 2, 512], BF16)
            for j in range(2):
                pzdt = psA.tile([128, 512], F32)
                mms = []
                for i in range(4):
                    js = slice(128 * j, 128 * (j + 1))
                    mms.append((wdh_t[:, i, js], zth[:, i, :]))
                    mms.append((wdl_t[:, i, js], zth[:, i, :]))
                    mms.append((wdh_t[:, i, js], ztl[:, i, :]))
                for q, (lw, rv) in enumerate(mms):
                    nc.tensor.matmul(out=pzdt[:], lhsT=lw, rhs=rv,
                                     start=(q == 0), stop=(q == len(mms) - 1))
                nc.scalar.copy(out=zdtf[:, j, :], in_=pzdt[:])
                nc.scalar.copy(out=zdth[:, j, :], in_=zdtf[:, j, :])
                nc.vector.tensor_sub(zdtl[:, j, :], zdtf[:, j, :], zdth[:, j, :])
                nc.scalar.dma_start(out=zdt_o[128 * j:128 * (j + 1), t0:t0 + 512],
                                    in_=zdtf[:, j, :])
                nc.scalar.activation(out=sqscr[:], in_=zdtf[:, j, :], func=SQUARE,
                                     accum_out=zd2_sb[:, 2 * b + j:2 * b + j + 1])

            if b == 0:
                # codebook constants land while block-0 proj runs
                nc.gpsimd.dma_start(out=cbh_t[:],
                                    in_=cbh.rearrange("(j p) k -> p j k", p=128))
                nc.gpsimd.dma_start(out=cbl_t[:],
                                    in_=cbl.rearrange("(j p) k -> p j k", p=128))
                c2h_b = bass.AP(tensor=c2h.tensor, offset=c2h.offset,
                                ap=[[0, 128], list(c2h.ap[-1])])
                nc.gpsimd.dma_start(out=c2h_t[:], in_=c2h_b)

            for tci in range(4):
                c = 4 * b + tci
                tsl = slice(128 * tci, 128 * (tci + 1))
                m_sb = mp.tile([128, K], F32)
                pm = psM.tile([128, 2, 512], F32)
                for h in range(2):
                    ks = slice(512 * h, 512 * (h + 1))
                    mms = []
                    for j in range(2):
                        mms.append((zdth[:, j, tsl], cbh_t[:, j, ks]))
                        mms.append((zdth[:, j, tsl], cbl_t[:, j, ks]))
                        mms.append((zdtl[:, j, tsl], cbh_t[:, j, ks]))
                    for q, (lw, rv) in enumerate(mms):
                        nc.tensor.matmul(out=pm[:, h, :], lhsT=lw, rhs=rv,
                                         start=(q == 0), stop=(q == len(mms) - 1))
                nc.vector.tensor_sub(m_sb[:],
                                     pm[:].rearrange("p a b -> p (a b)"),
                                     c2h_t[:])
                nc.vector.max(out=mv8_sb[:, c, :], in_=m_sb[:])
                nc.vector.max_index(out=idx8_sb[:, c, :], in_max=mv8_sb[:, c, :],
                                    in_values=m_sb[:])
                zq_t = zqp.tile([128, D_IN], F32)
                nc.gpsimd.indirect_dma_start(
                    out=zq_t[:], out_offset=None, in_=U[:],
                    in_offset=bass.IndirectOffsetOnAxis(ap=idx8_sb[:, c, 0:1], axis=0))
                nc.sync.dma_start(out=zq_o[t0 + 128 * tci:t0 + 128 * (tci + 1), :],
                                  in_=zq_t[:])

            cs = slice(4 * b, 4 * (b + 1))
            nc.scalar.dma_start(out=idx8_o[:, cs, :], in_=idx8_sb[:, cs, :])
            nc.scalar.dma_start(out=mv8_o[:, cs, :], in_=mv8_sb[:, cs, :])

        nc.scalar.dma_start(out=zd2_o[:], in_=zd2_sb[:])

    nc.compile()
    return nc


_NC_CACHE = []


def kernel(z_e, codebook, W_down, W_up, **run_kwargs):
    z_e = np.ascontiguousarray(np.asarray(z_e, dtype=np.float32)).reshape(TOK, D_IN)
    codebook = np.asarray(codebook, dtype=np.float32)
    W_down = np.asarray(W_down, dtype=np.float32)
    W_up = np.asarray(W_up, dtype=np.float32)

    # host prep: bf16 hi/lo splits + tiny precomputes
    zT = np.ascontiguousarray(z_e.T)                             # [512, 65536]
    zh, zl = _split_bf16(zT)
    wdh, wdl = _split_bf16(np.ascontiguousarray(W_down.T))       # [512, 256]
    cbh, cbl = _split_bf16(np.ascontiguousarray(codebook.T))     # [256, 1024]
    c2h = (0.5 * np.sum(codebook.astype(np.float64) ** 2, axis=1)
           ).astype(np.float32).reshape(1, K)
    U = (codebook.astype(np.float64) @ W_up.astype(np.float64).T
         ).astype(np.float32)                                    # [1024, 512]

    if not _NC_CACHE:
        _NC_CACHE.append(_build_nc())
    nc = _NC_CACHE[0]

    in_maps = []
    for ci in range(N_CORES):
        s = slice(ci * TPC, (ci + 1) * TPC)
        in_maps.append({
            "zh": np.ascontiguousarray(zh[:, s]),
            "zl": np.ascontiguousarray(zl[:, s]),
            "wdh": wdh, "wdl": wdl, "cbh": cbh, "cbl": cbl,
            "c2h": c2h, "U": U,
        })

    res = run_bass_kernel_spmd(nc, in_maps, core_ids=list(range(N_CORES)),
                               **run_kwargs)
    results = res.results

    z_q = np.empty((TOK, D_IN), dtype=np.float32)
    z_e_down = np.empty((TOK, D_EMB), dtype=np.float32)
    code = np.empty(TOK, dtype=np.int32)
    losses = np.empty(B, dtype=np.float32)
    for ci, r in enumerate(results):
        s = slice(ci * TPC, (ci + 1) * TPC)
        z_q[s] = r["zq_o"]
        z_e_down[s] = np.ascontiguousarray(r["zdt_o"].T)
        code[s] = np.ascontiguousarray(
            r["idx8_o"][:, :, 0].astype(np.int64).T).reshape(TPC).astype(np.int32)
        maxm = np.ascontiguousarray(r["mv8_o"][:, :, 0].T).reshape(TPC)
        zd2 = r["zd2_o"]
        for lb in range(2):   # 2 batches per core
            sum_mm = maxm[4096 * lb:4096 * (lb + 1)].astype(np.float64).sum()
            sum_z2 = zd2[:, 16 * lb:16 * (lb + 1)].astype(np.float64).sum()
            losses[2 * ci + lb] = np.float32(
                (sum_z2 - 2.0 * sum_mm) / (4096.0 * 256.0))

    z_q = z_q.reshape(B, T, D_IN)
    z_e_down = z_e_down.reshape(B, T, D_EMB)
    code = code.reshape(B, T)
    if run_kwargs:
        kernel.last_results = res
    return z_q, z_e_down, code, losses, losses.copy()
